# revision 8
# baseline (speedup 1.0000x reference)
"""Causal multi-head attention (B=2, S=2048, D=1024, H=16) on 8 TRN2 NeuronCores.

Sharding: core c handles batch b = c//4 and heads [4*(c%4), 4*(c%4)+4).
Each core computes its 4 heads' attention plus the partial w_o projection;
the host sums the 4 partials per batch (the "all-reduce after w_o") and
adds the w_o bias plus the (constant) v-bias-through-w_o row.

Compute dtype: bf16 matmul inputs with fp32 PSUM accumulation.

Layouts (per core, host-prepared):
  xT    [1024, 2048] bf16  x[b].T                    (d on partitions)
  wqk   [1024, 512]  bf16  cols = [k_h0..k_h3 | q_h0..q_h3] (64 each)
  wv    [1024, 256]  bf16  cols = [v_h0..v_h3]
  wo    [256, 1024]  bf16  w_o[:, head_cols].T
  bqk   [512, 1]     f32   per-feature bias, same col order as wqk
  masks [4, 128, 1024] bf16  causal 0/1 patterns for diagonal blocks at
                             k-offset delta = 0,128,256,384 inside a 512-wide
                             q tile; duplicated along free dim (two heads).

In-kernel dataflow (per core):
  qkvT = wqk.T @ x.T  -> k/q in [feature, seq] layout, paired heads per tile
  v    = x @ wv       -> natural [seq, feature] + a ones column per head
  scores_T[k,q] = k_h.T(dk x 128) @ q_h(dk x 512)   (two heads row-tiled)
  p = exp(scores/8)   (ACT, psum->sbuf, bf16) ; diag blocks masked via DVE
  av_T[dk+1, q] = [v_h | 1].T @ p                   (row 64 = softmax denom)
  rec = 1/den (DVE, bf16) ; bc = ones(64) (x) rec   (K=1 PE matmul -> PSUM)
  avn = av * bc                                     (DVE, -> bf16)
  y[s, o] = avn.T @ wo                              (partial, bf16 to HBM)

QKV / w_o work is interleaved into the attention stream at single-matmul
granularity so the scalar engine (exp) and PE stay concurrently busy.
"""

import numpy as np
import ml_dtypes

import concourse.bass as bass
import concourse.mybir as mybir
import concourse.tile as tile
from concourse.bass_utils import run_bass_kernel_spmd
from concourse.vector_clock import ScopedClock

BF16 = mybir.dt.bfloat16
F32 = mybir.dt.float32
NP_BF16 = ml_dtypes.bfloat16

B, S, D = 2, 2048, 1024
H, DK = 16, 64
HPC = 4            # heads per core
N_CORES = 8
S_TILE = 512       # q tile width (f32 psum bank)
K_BLK = 128        # k block (partition dim of transposed scores)


# ---------------------------------------------------------------------------
# Workaround: this walrus build rejects >1 sem-wait on the TileContext exit
# Drain. Redistribute the global-clock waits onto single-wait sync NOPs.
# ---------------------------------------------------------------------------
def _patched_drain_and_barrier(self, tick_clock, wait_clock):
    probe = self.nc.sync.nop()
    wait_clock.add_sem_waits(probe.ins, ScopedClock({None: tick_clock.global_clock}))
    si = probe.ins.sync_info
    waits = list(si.on_wait)
    probe.ins.sync_info = mybir.SyncInfo(on_wait=waits[:1], on_update=list(si.on_update))
    for w in waits[1:]:
        nop = self.nc.sync.nop()
        nop.ins.sync_info = mybir.SyncInfo(on_wait=[w], on_update=[])
    self.nc.sync.drain()

    self.nc.all_engine_barrier()
    assert self.sems is not None
    popped = self.nc._tile_sem_poison_stack.pop()
    assert popped is self._sem_poison
    self.nc.clear_and_free_semaphores(list(self.sems.allocated().values()))
    self.nc.all_engine_barrier()


tile.TileContext._drain_and_barrier = _patched_drain_and_barrier

# Enable walrus's LDWEIGHTS double-buffering (background weight buffer):
# without it every LDWEIGHTS serializes against the preceding MATMUL and
# back-to-back matmuls run at isolated latency (~375 ns vs ~220 ns at N=512).
# walrus rejects LDWEIGHTS instructions that carry semaphore waits when the
# opt is on, so _strip_ldweights_waits moves those waits onto PE NOPs.
_ENABLE_LDW_OPT = True
_CFG_SALT = f"cfg-ldw{int(_ENABLE_LDW_OPT)}-v2"

import concourse.bass_utils as _bu

_orig_run_command = _bu.run_command


def _run_command_with_flags(cmd, *args, **kwargs):
    if _ENABLE_LDW_OPT and isinstance(cmd, list):
        cmd = [
            "--enable-ldw-opt=true" if c == "--enable-ldw-opt=false" else c
            for c in cmd
        ]
    return _orig_run_command(cmd, *args, **kwargs)


_bu.run_command = _run_command_with_flags

_WAIT_LIMIT = 1


def _fuse_ldweights(nc: bass.Bass):
    """Re-fuse standalone InstLdweights into their paired InstMatmult
    (self-loading, mm.ldweights=True), merging sem waits/updates. walrus's
    --enable-ldw-opt rejects standalone LDWEIGHTS; for self-loading matmuls
    it emits the background-buffer double-buffered weight load itself."""
    n = 0
    for f in nc.m.functions:
        for bb in f.blocks:
            il = bb.instructions
            out = []
            pending = []  # unmatched ldweights, FIFO
            for inst in il:
                if isinstance(inst, mybir.InstLdweights):
                    pending.append(inst)
                    continue
                if isinstance(inst, mybir.InstMatmult) and pending:
                    lw = pending.pop(0)
                    assert lw.ins[0].memref == inst.ins[1].memref, (
                        f"ldweights {lw.name} paired with wrong matmul "
                        f"{inst.name}: {lw.ins[0].memref} != {inst.ins[1].memref}"
                    )
                    inst.ldweights = True
                    lsi = lw.sync_info
                    msi = inst.sync_info
                    inst.sync_info = mybir.SyncInfo(
                        on_wait=(list(lsi.on_wait) if lsi else [])
                        + (list(msi.on_wait) if msi else []),
                        on_update=(list(lsi.on_update) if lsi else [])
                        + (list(msi.on_update) if msi else []),
                    )
                    n += 1
                out.append(inst)
            assert not pending, f"unmatched ldweights: {[p.name for p in pending]}"
            bb.instructions = out
    return n


def _split_excess_waits(nc: bass.Bass, limit: int = _WAIT_LIMIT):
    """Walrus (this build) rejects instructions carrying more than a couple of
    sem waits. Move excess waits onto same-engine NOPs inserted just before."""
    n_split = 0
    for f in nc.m.functions:
        for bb in f.blocks:
            il = bb.instructions
            idx = 0
            while idx < len(il):
                inst = il[idx]
                si = inst.sync_info
                if si is not None and len(si.on_wait) > limit:
                    waits = list(si.on_wait)
                    pos = idx
                    for i in range(limit, len(waits), limit):
                        nop = mybir.InstNoOp(
                            name=f"{inst.name}_xw{i}", ins=[], outs=[]
                        )
                        nop.engine = inst.engine
                        nop.sync_info = mybir.SyncInfo(
                            on_wait=waits[i:i + limit], on_update=[]
                        )
                        il.insert(pos, nop)
                        pos += 1
                        idx += 1
                    inst.sync_info = mybir.SyncInfo(
                        on_wait=waits[:limit], on_update=list(si.on_update)
                    )
                    n_split += 1
                idx += 1
    return n_split


def build_attention_nc() -> bass.Bass:
    nc = bass.Bass("TRN2", target_bir_lowering=False, debug=False)

    xT_d = nc.dram_tensor("xT", [D, S], BF16, kind="ExternalInput").ap()
    wqk_d = nc.dram_tensor("wqk", [D, 8 * DK], BF16, kind="ExternalInput").ap()
    wv_d = nc.dram_tensor("wv", [D, 4 * DK], BF16, kind="ExternalInput").ap()
    wo_d = nc.dram_tensor("wo", [4 * DK, D], BF16, kind="ExternalInput").ap()
    bqk_d = nc.dram_tensor("bqk", [8 * DK, 1], F32, kind="ExternalInput").ap()
    masks_d = nc.dram_tensor("masks", [4, 128, 1024], BF16, kind="ExternalInput").ap()
    y_d = nc.dram_tensor("y", [S, D], BF16, kind="ExternalOutput").ap()

    n_kt = D // 128          # 8 contraction tiles over d
    n_st = S // 128          # 16 seq tiles of 128
    n_qt = S // S_TILE       # 4 q tiles of 512
    AV_LAG = 2               # AV trails exp by this many blocks in the stream

    from contextlib import ExitStack

    with tile.TileContext(nc) as tc, ExitStack() as stack:
        const = stack.enter_context(tc.tile_pool(name="const", bufs=1))
        xpool = stack.enter_context(tc.tile_pool(name="xp", bufs=1))
        kqpool = stack.enter_context(tc.tile_pool(name="kqp", bufs=1))
        vpool = stack.enter_context(tc.tile_pool(name="vp", bufs=1))
        avnpool = stack.enter_context(tc.tile_pool(name="avnp", bufs=1))
        ppool = stack.enter_context(tc.tile_pool(name="pp", bufs=24))
        spool = stack.enter_context(tc.tile_pool(name="sp", bufs=4))
        ypool = stack.enter_context(tc.tile_pool(name="yp", bufs=2))
        avsb = stack.enter_context(tc.tile_pool(name="avsb", bufs=4))
        # PSUM budget (8 banks): scores/y 2x[128,1024]=4, qkv 2 (qk+v tags), av/bc 2.
        sc_ps = stack.enter_context(tc.tile_pool(name="sc_ps", bufs=2, space="PSUM"))
        qkv_ps = stack.enter_context(tc.tile_pool(name="qkv_ps", bufs=1, space="PSUM"))
        av_ps = stack.enter_context(tc.tile_pool(name="av_ps", bufs=2, space="PSUM"))

        # --- resident loads (ordered so QKV compute can start early) ----
        xT, wqk, wv = [], [], []
        for i in range(n_kt):
            t = xpool.tile([128, S], BF16, tag=f"xT{i}", name=f"xT{i}")
            xT.append(t)
        for i in range(n_kt):
            w1 = const.tile([128, 8 * DK], BF16, tag=f"wqk{i}", name=f"wqk{i}")
            nc.scalar.dma_start(out=w1, in_=wqk_d[i * 128:(i + 1) * 128, :])
            wqk.append(w1)
            w2 = const.tile([128, 4 * DK], BF16, tag=f"wv{i}", name=f"wv{i}")
            nc.scalar.dma_start(out=w2, in_=wv_d[i * 128:(i + 1) * 128, :])
            wv.append(w2)
            nc.sync.dma_start(
                out=xT[i][:, 0:S_TILE], in_=xT_d[i * 128:(i + 1) * 128, 0:S_TILE]
            )
        for sq in range(1, n_qt):
            for i in range(n_kt):
                nc.sync.dma_start(
                    out=xT[i][:, sq * S_TILE:(sq + 1) * S_TILE],
                    in_=xT_d[i * 128:(i + 1) * 128, sq * S_TILE:(sq + 1) * S_TILE],
                )
        bqk = []
        for i in range(4):
            t = const.tile([128, 1], F32, tag=f"bqk{i}", name=f"bqk{i}")
            nc.scalar.dma_start(out=t, in_=bqk_d[i * 128:(i + 1) * 128, :])
            bqk.append(t)
        masks = []
        for i in range(4):
            t = const.tile([128, 1024], BF16, tag=f"mask{i}", name=f"mask{i}")
            nc.scalar.dma_start(out=t, in_=masks_d[i])
            masks.append(t)
        wo = []
        for i in range(2):
            t = const.tile([128, D], BF16, tag=f"wo{i}", name=f"wo{i}")
            nc.scalar.dma_start(out=t, in_=wo_d[i * 128:(i + 1) * 128, :])
            wo.append(t)
        ones_row = const.tile([1, DK], BF16, tag="ones", name="ones")
        nc.vector.memset(ones_row, 1.0)

        # kq[m][f, s]: m=0 -> k heads(0,1); 1 -> k heads(2,3); 2 -> q(0,1); 3 -> q(2,3)
        kq = [kqpool.tile([128, S], BF16, tag=f"kq{m}", name=f"kq{m}") for m in range(4)]
        # v_sb[st][128, 4*65]: per head h: cols [h*65, h*65+64) = v, col h*65+64 = 1.0
        v_sb = [vpool.tile([128, HPC * (DK + 1)], BF16, tag=f"v{st}", name=f"v{st}")
                for st in range(n_st)]
        # avn[f2][f, s]: f2=0 -> heads (0,1); f2=1 -> heads (2,3)
        avn = [avnpool.tile([128, S], BF16, tag=f"avn{f2}", name=f"avn{f2}")
               for f2 in range(2)]

        # ---- QKV / w_o emission, at single-matmul micro-step granularity ----
        def kq_steps(m, sq):
            st8 = {}

            def mm(kt):
                def f():
                    if kt == 0:
                        st8['ps'] = qkv_ps.tile([128, S_TILE], F32, tag="qkps",
                                                name="qkps")
                    nc.tensor.matmul(
                        st8['ps'],
                        lhsT=wqk[kt][:, m * 128:(m + 1) * 128],
                        rhs=xT[kt][:, sq * S_TILE:(sq + 1) * S_TILE],
                        start=(kt == 0),
                        stop=(kt == n_kt - 1),
                    )
                return f

            def fin():
                nc.vector.tensor_scalar_add(
                    kq[m][:, sq * S_TILE:(sq + 1) * S_TILE], st8['ps'], bqk[m]
                )

            return [mm(kt) for kt in range(n_kt)] + [fin]

        def v_steps(st):
            st8 = {}

            def mm(kt):
                def f():
                    if kt == 0:
                        nc.vector.memset(v_sb[st], 1.0)
                        st8['ps'] = qkv_ps.tile([128, HPC * DK], F32, tag="vps",
                                                name="vps")
                    nc.tensor.matmul(
                        st8['ps'],
                        lhsT=xT[kt][:, st * 128:(st + 1) * 128],
                        rhs=wv[kt],
                        start=(kt == 0),
                        stop=(kt == n_kt - 1),
                    )
                return f

            def cp(h0):
                def f():
                    for h in (h0, h0 + 1):
                        nc.vector.tensor_copy(
                            out=v_sb[st][:, h * (DK + 1):h * (DK + 1) + DK],
                            in_=st8['ps'][:, h * DK:(h + 1) * DK],
                        )
                return f

            return [mm(kt) for kt in range(n_kt)] + [cp(0), cp(2)]

        def wo_steps(st):
            st8 = {}

            def mm(oh, f2):
                def f():
                    if oh == 0 and f2 == 0:
                        st8['yp'] = sc_ps.tile([128, D], F32, tag="scps", name="yps")
                    nc.tensor.matmul(
                        st8['yp'][:, oh * 512:(oh + 1) * 512],
                        lhsT=avn[f2][:, st * 128:(st + 1) * 128],
                        rhs=wo[f2][:, oh * 512:(oh + 1) * 512],
                        start=(f2 == 0),
                        stop=(f2 == 1),
                    )
                return f

            def out():
                y_sb = ypool.tile([128, D], BF16, tag="ysb", name="ysb")
                nc.vector.tensor_copy(out=y_sb, in_=st8['yp'])
                nc.sync.dma_start(out=y_d[st * 128:(st + 1) * 128, :], in_=y_sb)

            return [mm(oh, f2) for oh in range(2) for f2 in range(2)] + [out]

        def qkv_round(sq):
            steps = []
            for m in (0, 2, 1, 3):
                steps.extend(kq_steps(m, sq))
            for st in range(4 * sq, 4 * sq + 4):
                steps.extend(v_steps(st))
            return steps

        def normalize(t, hp, av_t):
            """Move av+den off PSUM, then avn = av * (1/den) via a K=1
            PE-broadcast of the reciprocal row (no DRAM bounces)."""
            av_c = []
            for i in range(2):
                c = avsb.tile([DK + 1, S_TILE], F32, tag="avc", name="avc")
                nc.vector.tensor_copy(out=c, in_=av_t[i][0:DK + 1, :])
                av_c.append(c)
            for i in range(2):
                rec = spool.tile([1, S_TILE], BF16, tag="rec", name="rec")
                with nc.allow_low_precision(reason="bf16 softmax denom reciprocal"):
                    nc.vector.reciprocal(rec, av_c[i][DK:DK + 1, :])
                bc = av_ps.tile([DK, S_TILE], F32, tag="avps", name="bcps")
                nc.tensor.matmul(
                    bc, lhsT=ones_row, rhs=rec, start=True, stop=True
                )
                if i == 0:
                    dst = avn[hp][0:DK, t * S_TILE:(t + 1) * S_TILE]
                    nc.vector.tensor_mul(dst, av_c[i][0:DK, :], bc)
                else:
                    tmp = spool.tile([DK, S_TILE], BF16, tag="avtmp", name="avtmp")
                    nc.vector.tensor_mul(tmp, av_c[i][0:DK, :], bc)
                    nc.sync.dma_start(
                        out=avn[hp][64:128, t * S_TILE:(t + 1) * S_TILE],
                        in_=tmp,
                    )

        def attention_tile(t, steps):
            """Emit attention for q-tile t, interleaving `steps` (QKV matmuls
            of the next round, w_o of the previous tile) into the stream. AV
            matmuls trail their exp by AV_LAG blocks so the in-order PE
            stream never parks on an unfinished exp."""
            nblk = 4 * t + 4
            nslots = 2 * nblk
            total = len(steps)
            done = [0]
            slot = [0]

            def run_share():
                slot[0] += 1
                want = min(total, (total * slot[0]) // nslots)
                while done[0] < want:
                    steps[done[0]]()
                    done[0] += 1

            for hp in range(2):
                kt2 = kq[hp]
                qt2 = kq[2 + hp]
                av_t = [av_ps.tile([128, S_TILE], F32, tag="avps", name="avps")
                        for _ in range(2)]
                pend = []

                def emit_av(blk, p):
                    for i in range(2):
                        h = 2 * hp + i
                        nc.tensor.matmul(
                            av_t[i][0:DK + 1, :],
                            lhsT=v_sb[blk][:, h * (DK + 1):(h + 1) * (DK + 1)],
                            rhs=p[:, i * S_TILE:(i + 1) * S_TILE],
                            start=(blk == 0),
                            stop=(blk == nblk - 1),
                        )

                for blk in range(nblk):
                    sc = sc_ps.tile([128, 2 * S_TILE], F32, tag="scps", name="scps")
                    for i in range(2):  # head A / head B, row-tiled pair
                        nc.tensor.matmul(
                            sc[:, i * S_TILE:(i + 1) * S_TILE],
                            lhsT=kt2[i * 64:(i + 1) * 64, blk * K_BLK:(blk + 1) * K_BLK],
                            rhs=qt2[i * 64:(i + 1) * 64, t * S_TILE:(t + 1) * S_TILE],
                            start=True,
                            stop=True,
                            tile_position=(i * 64, 0),
                        )
                    p = ppool.tile([128, 2 * S_TILE], BF16, tag="p", name="p")
                    nc.scalar.activation(p, sc, mybir.ActivationFunctionType.Exp,
                                         scale=0.125)
                    dd = blk - 4 * t
                    if dd >= 0:       # diagonal block: apply causal 0/1 mask
                        nc.vector.tensor_mul(p, p, masks[dd])
                    pend.append((blk, p))
                    run_share()
                    if len(pend) > AV_LAG:
                        emit_av(*pend.pop(0))
                while pend:
                    run_share()
                    emit_av(*pend.pop(0))
                normalize(t, hp, av_t)
            while done[0] < total:
                steps[done[0]]()
                done[0] += 1

        for step in qkv_round(0):
            step()
        for t in range(n_qt):
            steps = qkv_round(t + 1) if t + 1 < n_qt else []
            if t > 0:
                steps = [s for st in range(4 * (t - 1), 4 * t)
                         for s in wo_steps(st)] + steps
            attention_tile(t, steps)
        for st in range(4 * (n_qt - 1), n_st):
            for s in wo_steps(st):
                s()

    if _ENABLE_LDW_OPT:
        _fuse_ldweights(nc)
    _split_excess_waits(nc)
    salt = mybir.InstNoOp(name=f"salt_{_CFG_SALT}", ins=[], outs=[])
    salt.engine = mybir.EngineType.SP
    nc.m.functions[0].blocks[0].instructions.insert(0, salt)
    return nc


_CACHED_NC = None


def _get_nc():
    global _CACHED_NC
    if _CACHED_NC is None:
        _CACHED_NC = build_attention_nc()
    return _CACHED_NC


def _prep_core_inputs(x, mask, w_qkv_w, w_qkv_b, w_o_w, w_o_b, core):
    b = core // 4
    hg = core % 4
    heads = [hg * HPC + h for h in range(HPC)]

    xT = np.ascontiguousarray(x[b].T).astype(NP_BF16)

    def rows(sec, h):  # q=0, k=1, v=2
        base = sec * D + h * DK
        return slice(base, base + DK)

    wqk_rows = np.concatenate(
        [w_qkv_w[rows(1, h)] for h in heads] + [w_qkv_w[rows(0, h)] for h in heads],
        axis=0,
    )  # [512, 1024]
    wqk = np.ascontiguousarray(wqk_rows.T).astype(NP_BF16)

    wv_rows = np.concatenate([w_qkv_w[rows(2, h)] for h in heads], axis=0)
    wv = np.ascontiguousarray(wv_rows.T).astype(NP_BF16)

    wo = np.ascontiguousarray(
        w_o_w[:, hg * HPC * DK:(hg + 1) * HPC * DK].T
    ).astype(NP_BF16)

    bqk = np.concatenate(
        [w_qkv_b[rows(1, h)] for h in heads] + [w_qkv_b[rows(0, h)] for h in heads]
    ).astype(np.float32)[:, None]

    # Diagonal-block mask patterns from the provided mask tensor.
    m2d = np.asarray(mask[0, 0])
    q0 = S - S_TILE
    pats = []
    for dd in range(4):
        k0 = q0 + dd * K_BLK
        pat = m2d[q0:q0 + S_TILE, k0:k0 + K_BLK].T.astype(np.float32)  # [128, 512]
        pats.append(np.concatenate([pat, pat], axis=1))               # [128, 1024]
    masks_np = np.stack(pats).astype(NP_BF16)

    return {
        "xT": xT, "wqk": wqk, "wv": wv, "wo": wo,
        "bqk": bqk, "masks": masks_np,
    }


def kernel(x, mask, w_qkv_w, w_qkv_b, w_o_w, w_o_b, _profile=False):
    x = np.asarray(x, np.float32)
    w_qkv_w = np.asarray(w_qkv_w, np.float32)
    w_qkv_b = np.asarray(w_qkv_b, np.float32)
    w_o_w = np.asarray(w_o_w, np.float32)
    w_o_b = np.asarray(w_o_b, np.float32)

    nc = _get_nc()
    in_maps = [
        _prep_core_inputs(x, mask, w_qkv_w, w_qkv_b, w_o_w, w_o_b, core=c)
        for c in range(N_CORES)
    ]
    res = run_bass_kernel_spmd(
        nc, in_maps, core_ids=list(range(N_CORES)), trace=_profile
    )
    y = np.zeros((B, S, D), np.float32)
    for c in range(N_CORES):
        y[c // 4] += np.asarray(res.results[c]["y"], np.float32)
    # v-bias flows through w_o as a constant row: y += w_o @ b_v + b_o.
    y += (w_o_w @ w_qkv_b[2 * D:3 * D] + w_o_b)[None, None, :]
    if _profile:
        return y, res
    return y


# revision 12
# speedup vs baseline: 1.1304x; 1.1304x over previous
"""Causal multi-head attention (B=2, S=2048, D=1024, H=16) on 8 TRN2 NeuronCores.

Sharding: core c handles batch b = c//4 and heads [4*(c%4), 4*(c%4)+4).
Each core computes its 4 heads' attention plus the partial w_o projection;
the host sums the 4 partials per batch (the "all-reduce after w_o") and
adds the w_o bias plus the (constant) v-bias-through-w_o row.

Compute dtype: bf16 matmul inputs with fp32 PSUM accumulation.

Layouts (per core, host-prepared):
  xT    [1024, 2048] bf16  x[b].T                    (d on partitions)
  wqk   [1024, 512]  bf16  cols = [k_h0..k_h3 | q_h0..q_h3] (64 each)
  wv    [1024, 256]  bf16  cols = [v_h0..v_h3]
  wo    [256, 1024]  bf16  w_o[:, head_cols].T
  bqk   [512, 1]     f32   per-feature bias, same col order as wqk
  masks [4, 128, 1024] bf16  causal 0/1 patterns for diagonal blocks at
                             k-offset delta = 0,128,256,384 inside a 512-wide
                             q tile; duplicated along free dim (two heads).

In-kernel dataflow (per core):
  qkvT = wqk.T @ x.T  -> k/q in [feature, seq] layout, paired heads per tile
  v    = x @ wv       -> natural [seq, feature] + a ones column per head
  scores_T[k,q] = k_h.T(dk x 128) @ q_h(dk x 512)   (two heads row-tiled)
  p = exp(scores/8)   (ACT, psum->sbuf, bf16) ; diag blocks masked via DVE
  av_T[dk+1, q] = [v_h | 1].T @ p                   (row 64 = softmax denom)
  rec = 1/den (DVE, bf16) ; bc = ones(64) (x) rec   (K=1 PE matmul -> PSUM)
  avn = av * bc                                     (DVE, -> bf16)
  y[s, o] = avn.T @ wo                              (partial, bf16 to HBM)

QKV / w_o work is interleaved into the attention stream at single-matmul
granularity so the scalar engine (exp) and PE stay concurrently busy.
"""

import numpy as np
import ml_dtypes

import concourse.bass as bass
import concourse.mybir as mybir
import concourse.tile as tile
from concourse.bass_utils import run_bass_kernel_spmd
from concourse.vector_clock import ScopedClock

BF16 = mybir.dt.bfloat16
F32 = mybir.dt.float32
NP_BF16 = ml_dtypes.bfloat16

B, S, D = 2, 2048, 1024
H, DK = 16, 64
HPC = 4            # heads per core
N_CORES = 8
S_TILE = 512       # q tile width (f32 psum bank)
K_BLK = 128        # k block (partition dim of transposed scores)


# ---------------------------------------------------------------------------
# Workaround: this walrus build rejects >1 sem-wait on the TileContext exit
# Drain. Redistribute the global-clock waits onto single-wait sync NOPs.
# ---------------------------------------------------------------------------
def _patched_drain_and_barrier(self, tick_clock, wait_clock):
    probe = self.nc.sync.nop()
    wait_clock.add_sem_waits(probe.ins, ScopedClock({None: tick_clock.global_clock}))
    si = probe.ins.sync_info
    waits = list(si.on_wait)
    probe.ins.sync_info = mybir.SyncInfo(on_wait=waits[:1], on_update=list(si.on_update))
    for w in waits[1:]:
        nop = self.nc.sync.nop()
        nop.ins.sync_info = mybir.SyncInfo(on_wait=[w], on_update=[])
    self.nc.sync.drain()

    self.nc.all_engine_barrier()
    assert self.sems is not None
    popped = self.nc._tile_sem_poison_stack.pop()
    assert popped is self._sem_poison
    self.nc.clear_and_free_semaphores(list(self.sems.allocated().values()))
    self.nc.all_engine_barrier()


tile.TileContext._drain_and_barrier = _patched_drain_and_barrier

# Enable walrus's LDWEIGHTS double-buffering (background weight buffer):
# without it every LDWEIGHTS serializes against the preceding MATMUL and
# back-to-back matmuls run at isolated latency (~375 ns vs ~220 ns at N=512).
# walrus rejects LDWEIGHTS instructions that carry semaphore waits when the
# opt is on, so _strip_ldweights_waits moves those waits onto PE NOPs.
_ENABLE_LDW_OPT = True
_CFG_SALT = f"cfg-ldw{int(_ENABLE_LDW_OPT)}-v3"

import concourse.bass_utils as _bu

_orig_run_command = _bu.run_command


def _run_command_with_flags(cmd, *args, **kwargs):
    if _ENABLE_LDW_OPT and isinstance(cmd, list):
        cmd = [
            "--enable-ldw-opt=true" if c == "--enable-ldw-opt=false" else c
            for c in cmd
        ]
    return _orig_run_command(cmd, *args, **kwargs)


_bu.run_command = _run_command_with_flags

_WAIT_LIMIT = 1


def _fuse_ldweights(nc: bass.Bass):
    """Re-fuse standalone InstLdweights into their paired InstMatmult
    (self-loading, mm.ldweights=True), merging sem waits/updates. walrus's
    --enable-ldw-opt rejects standalone LDWEIGHTS; for self-loading matmuls
    it emits the background-buffer double-buffered weight load itself."""
    n = 0
    for f in nc.m.functions:
        for bb in f.blocks:
            il = bb.instructions
            out = []
            pending = []  # unmatched ldweights, FIFO
            for inst in il:
                if isinstance(inst, mybir.InstLdweights):
                    pending.append(inst)
                    continue
                if isinstance(inst, mybir.InstMatmult) and pending:
                    lw = pending.pop(0)
                    assert lw.ins[0].memref == inst.ins[1].memref, (
                        f"ldweights {lw.name} paired with wrong matmul "
                        f"{inst.name}: {lw.ins[0].memref} != {inst.ins[1].memref}"
                    )
                    inst.ldweights = True
                    lsi = lw.sync_info
                    msi = inst.sync_info
                    inst.sync_info = mybir.SyncInfo(
                        on_wait=(list(lsi.on_wait) if lsi else [])
                        + (list(msi.on_wait) if msi else []),
                        on_update=(list(lsi.on_update) if lsi else [])
                        + (list(msi.on_update) if msi else []),
                    )
                    n += 1
                out.append(inst)
            assert not pending, f"unmatched ldweights: {[p.name for p in pending]}"
            bb.instructions = out
    return n


def _split_excess_waits(nc: bass.Bass, limit: int = _WAIT_LIMIT):
    """Walrus (this build) rejects instructions carrying more than a couple of
    sem waits. Move excess waits onto same-engine NOPs inserted just before."""
    n_split = 0
    for f in nc.m.functions:
        for bb in f.blocks:
            il = bb.instructions
            idx = 0
            while idx < len(il):
                inst = il[idx]
                si = inst.sync_info
                if si is not None and len(si.on_wait) > limit:
                    waits = list(si.on_wait)
                    pos = idx
                    for i in range(limit, len(waits), limit):
                        nop = mybir.InstNoOp(
                            name=f"{inst.name}_xw{i}", ins=[], outs=[]
                        )
                        nop.engine = inst.engine
                        nop.sync_info = mybir.SyncInfo(
                            on_wait=waits[i:i + limit], on_update=[]
                        )
                        il.insert(pos, nop)
                        pos += 1
                        idx += 1
                    inst.sync_info = mybir.SyncInfo(
                        on_wait=waits[:limit], on_update=list(si.on_update)
                    )
                    n_split += 1
                idx += 1
    return n_split


def build_attention_nc() -> bass.Bass:
    nc = bass.Bass("TRN2", target_bir_lowering=False, debug=False)

    xT_d = nc.dram_tensor("xT", [D, S], BF16, kind="ExternalInput").ap()
    wqk_d = nc.dram_tensor("wqk", [D, 8 * DK], BF16, kind="ExternalInput").ap()
    wv_d = nc.dram_tensor("wv", [D, 4 * DK], BF16, kind="ExternalInput").ap()
    wo_d = nc.dram_tensor("wo", [4 * DK, D], BF16, kind="ExternalInput").ap()
    bqk_d = nc.dram_tensor("bqk", [8 * DK, 1], F32, kind="ExternalInput").ap()
    tri_d = nc.dram_tensor("tri", [128, 128], BF16, kind="ExternalInput").ap()
    y_d = nc.dram_tensor("y", [S, D], BF16, kind="ExternalOutput").ap()

    n_kt = D // 128          # 8 contraction tiles over d
    n_st = S // 128          # 16 seq tiles of 128
    n_qt = S // S_TILE       # 4 q tiles of 512
    AV_LAG = 2               # AV trails exp by this many blocks in the stream

    from contextlib import ExitStack

    with tile.TileContext(nc) as tc, ExitStack() as stack:
        const = stack.enter_context(tc.tile_pool(name="const", bufs=1))
        xpool = stack.enter_context(tc.tile_pool(name="xp", bufs=1))
        kqpool = stack.enter_context(tc.tile_pool(name="kqp", bufs=1))
        vpool = stack.enter_context(tc.tile_pool(name="vp", bufs=1))
        avnpool = stack.enter_context(tc.tile_pool(name="avnp", bufs=1))
        ppool = stack.enter_context(tc.tile_pool(name="pp", bufs=24))
        spool = stack.enter_context(tc.tile_pool(name="sp", bufs=4))
        ypool = stack.enter_context(tc.tile_pool(name="yp", bufs=2))
        avsb = stack.enter_context(tc.tile_pool(name="avsb", bufs=4))
        # PSUM budget (8 banks): scores/y 2x[128,1024]=4, qkv 2 (shared tag), av 2.
        sc_ps = stack.enter_context(tc.tile_pool(name="sc_ps", bufs=2, space="PSUM"))
        qkv_ps = stack.enter_context(tc.tile_pool(name="qkv_ps", bufs=2, space="PSUM"))
        av_ps = stack.enter_context(tc.tile_pool(name="av_ps", bufs=2, space="PSUM"))

        # --- resident loads (ordered so QKV compute can start early) ----
        xT, wqk, wv = [], [], []
        for i in range(n_kt):
            t = xpool.tile([128, S], BF16, tag=f"xT{i}", name=f"xT{i}")
            xT.append(t)
        for i in range(n_kt):
            w1 = const.tile([128, 8 * DK], BF16, tag=f"wqk{i}", name=f"wqk{i}")
            nc.gpsimd.dma_start(out=w1, in_=wqk_d[i * 128:(i + 1) * 128, :])
            wqk.append(w1)
            w2 = const.tile([128, 4 * DK], BF16, tag=f"wv{i}", name=f"wv{i}")
            nc.gpsimd.dma_start(out=w2, in_=wv_d[i * 128:(i + 1) * 128, :])
            wv.append(w2)
            nc.sync.dma_start(
                out=xT[i][:, 0:S_TILE], in_=xT_d[i * 128:(i + 1) * 128, 0:S_TILE]
            )
        for sq in range(1, n_qt):
            for i in range(n_kt):
                nc.sync.dma_start(
                    out=xT[i][:, sq * S_TILE:(sq + 1) * S_TILE],
                    in_=xT_d[i * 128:(i + 1) * 128, sq * S_TILE:(sq + 1) * S_TILE],
                )
        bqk = []
        for i in range(4):
            t = const.tile([128, 1], F32, tag=f"bqk{i}", name=f"bqk{i}")
            nc.gpsimd.dma_start(out=t, in_=bqk_d[i * 128:(i + 1) * 128, :])
            bqk.append(t)
        tri = const.tile([128, 128], BF16, tag="tri", name="tri")
        nc.gpsimd.dma_start(out=tri, in_=tri_d)
        wo = []
        for i in range(2):
            t = const.tile([128, D], BF16, tag=f"wo{i}", name=f"wo{i}")
            nc.gpsimd.dma_start(out=t, in_=wo_d[i * 128:(i + 1) * 128, :])
            wo.append(t)
        ones_row = const.tile([1, DK], BF16, tag="ones", name="ones")
        nc.vector.memset(ones_row, 1.0)
        # kq[m][f, s]: m=0 -> k heads(0,1); 1 -> k heads(2,3); 2 -> q(0,1); 3 -> q(2,3)
        kq = [kqpool.tile([128, S], BF16, tag=f"kq{m}", name=f"kq{m}") for m in range(4)]
        # v_sb[st][128, 4*65]: per head h: cols [h*65, h*65+64) = v, col h*65+64 = 1.0
        v_sb = [vpool.tile([128, HPC * (DK + 1)], BF16, tag=f"v{st}", name=f"v{st}")
                for st in range(n_st)]
        # avn[f2][f, s]: f2=0 -> heads (0,1); f2=1 -> heads (2,3)
        avn = [avnpool.tile([128, S], BF16, tag=f"avn{f2}", name=f"avn{f2}")
               for f2 in range(2)]

        # ---- QKV / w_o emission, at single-matmul micro-step granularity ----
        def kq_steps(m, sq):
            st8 = {}

            def mm(kt):
                def f():
                    if kt == 0:
                        st8['ps'] = qkv_ps.tile([128, S_TILE], F32, tag="qkps",
                                                name="qkps")
                    nc.tensor.matmul(
                        st8['ps'],
                        lhsT=wqk[kt][:, m * 128:(m + 1) * 128],
                        rhs=xT[kt][:, sq * S_TILE:(sq + 1) * S_TILE],
                        start=(kt == 0),
                        stop=(kt == n_kt - 1),
                    )
                return f

            def fin():
                nc.vector.tensor_scalar_add(
                    kq[m][:, sq * S_TILE:(sq + 1) * S_TILE], st8['ps'], bqk[m]
                )

            return [mm(kt) for kt in range(n_kt)] + [fin]

        def v_steps(st):
            st8 = {}

            def mm(kt):
                def f():
                    if kt == 0:
                        nc.gpsimd.memset(v_sb[st], 1.0)
                        st8['ps'] = qkv_ps.tile([128, HPC * DK], F32, tag="qkps",
                                                name="vps")
                    nc.tensor.matmul(
                        st8['ps'],
                        lhsT=xT[kt][:, st * 128:(st + 1) * 128],
                        rhs=wv[kt],
                        start=(kt == 0),
                        stop=(kt == n_kt - 1),
                    )
                return f

            def cp(h0):
                def f():
                    for h in (h0, h0 + 1):
                        nc.vector.tensor_copy(
                            out=v_sb[st][:, h * (DK + 1):h * (DK + 1) + DK],
                            in_=st8['ps'][:, h * DK:(h + 1) * DK],
                        )
                return f

            return [mm(kt) for kt in range(n_kt)] + [cp(0), cp(2)]

        def wo_steps(st):
            st8 = {}

            def mm(oh, f2):
                def f():
                    if oh == 0 and f2 == 0:
                        st8['yp'] = sc_ps.tile([128, D], F32, tag="scps", name="yps")
                    nc.tensor.matmul(
                        st8['yp'][:, oh * 512:(oh + 1) * 512],
                        lhsT=avn[f2][:, st * 128:(st + 1) * 128],
                        rhs=wo[f2][:, oh * 512:(oh + 1) * 512],
                        start=(f2 == 0),
                        stop=(f2 == 1),
                    )
                return f

            def out():
                y_sb = ypool.tile([128, D], BF16, tag="ysb", name="ysb")
                nc.vector.tensor_copy(out=y_sb, in_=st8['yp'])
                nc.sync.dma_start(out=y_d[st * 128:(st + 1) * 128, :], in_=y_sb)

            return [mm(oh, f2) for oh in range(2) for f2 in range(2)] + [out]

        def qkv_round(sq):
            steps = []
            for m in (0, 2, 1, 3):
                steps.extend(kq_steps(m, sq))
            for st in range(4 * sq, 4 * sq + 4):
                steps.extend(v_steps(st))
            return steps

        def normalize(t, hp, av_t):
            """Move av+den off PSUM, then avn = av * (1/den). The reciprocal
            runs at [128, 4] (DVE cost ~ free dim) via SBUF->SBUF reshape
            DMAs; broadcast across the 64 feature rows via a K=1 PE matmul
            against a ones row (no DRAM bounces)."""
            av_c = []
            for i in range(2):
                c = avsb.tile([DK + 1, S_TILE], F32, tag="avc", name="avc")
                nc.vector.tensor_copy(out=c, in_=av_t[i][0:DK + 1, :])
                av_c.append(c)
            for i in range(2):
                den2 = spool.tile([128, 4], F32, tag="den2", name="den2")
                nc.gpsimd.dma_start(out=den2, in_=av_c[i][DK:DK + 1, :])
                rec2 = spool.tile([128, 4], BF16, tag="rec2", name="rec2")
                with nc.allow_low_precision(reason="bf16 softmax denom recip"):
                    nc.vector.reciprocal(rec2, den2)
                rec = spool.tile([1, S_TILE], BF16, tag="rec", name="rec")
                nc.gpsimd.dma_start(out=rec, in_=rec2)
                bc = av_ps.tile([DK, S_TILE], F32, tag="avps", name="bcps")
                nc.tensor.matmul(bc, lhsT=ones_row, rhs=rec, start=True,
                                 stop=True)
                if i == 0:
                    dst = avn[hp][0:DK, t * S_TILE:(t + 1) * S_TILE]
                    nc.vector.tensor_mul(dst, av_c[i][0:DK, :], bc)
                else:
                    tmp = spool.tile([DK, S_TILE], BF16, tag="avtmp", name="avtmp")
                    nc.vector.tensor_mul(tmp, av_c[i][0:DK, :], bc)
                    nc.sync.dma_start(
                        out=avn[hp][64:128, t * S_TILE:(t + 1) * S_TILE],
                        in_=tmp,
                    )

        def attention_tile(t, steps):
            """Emit attention for q-tile t, interleaving `steps` (QKV matmuls
            of the next round, w_o of the previous tile) into the stream. AV
            matmuls trail their exp by AV_LAG blocks so the in-order PE
            stream never parks on an unfinished exp."""
            nblk = 4 * t + 4
            nslots = 2 * nblk
            total = len(steps)
            done = [0]
            slot = [0]

            def run_share():
                slot[0] += 1
                want = min(total, (total * slot[0]) // nslots)
                while done[0] < want:
                    steps[done[0]]()
                    done[0] += 1

            for hp in range(2):
                kt2 = kq[hp]
                qt2 = kq[2 + hp]
                av_t = [av_ps.tile([128, S_TILE], F32, tag="avps", name="avps")
                        for _ in range(2)]
                pend = []

                def emit_av(blk, p):
                    q0 = max(0, blk - 4 * t) * K_BLK
                    for i in range(2):
                        h = 2 * hp + i
                        nc.tensor.matmul(
                            av_t[i][0:DK + 1, q0:],
                            lhsT=v_sb[blk][:, h * (DK + 1):(h + 1) * (DK + 1)],
                            rhs=p[:, i * S_TILE + q0:(i + 1) * S_TILE],
                            start=(blk == 0),
                            stop=(blk == nblk - 1),
                        )

                for blk in range(nblk):
                    dd = blk - 4 * t
                    q0 = max(0, dd) * K_BLK   # first valid q col (causal)
                    sc = sc_ps.tile([128, 2 * S_TILE], F32, tag="scps", name="scps")
                    for i in range(2):  # head A / head B, row-tiled pair
                        nc.tensor.matmul(
                            sc[:, i * S_TILE + q0:(i + 1) * S_TILE],
                            lhsT=kt2[i * 64:(i + 1) * 64, blk * K_BLK:(blk + 1) * K_BLK],
                            rhs=qt2[i * 64:(i + 1) * 64,
                                    t * S_TILE + q0:(t + 1) * S_TILE],
                            start=True,
                            stop=True,
                            tile_position=(i * 64, 0),
                        )
                    p = ppool.tile([128, 2 * S_TILE], BF16, tag="p", name="p")
                    if q0 == 0:
                        nc.scalar.activation(p, sc,
                                             mybir.ActivationFunctionType.Exp,
                                             scale=0.125)
                    else:
                        for i in range(2):
                            nc.scalar.activation(
                                p[:, i * S_TILE + q0:(i + 1) * S_TILE],
                                sc[:, i * S_TILE + q0:(i + 1) * S_TILE],
                                mybir.ActivationFunctionType.Exp, scale=0.125)
                    if dd >= 0:       # diagonal 128-col slice: triangle mask
                        for i in range(2):
                            sl = slice(i * S_TILE + q0, i * S_TILE + q0 + K_BLK)
                            nc.gpsimd.tensor_mul(p[:, sl], p[:, sl], tri)
                    pend.append((blk, p))
                    run_share()
                    if len(pend) > AV_LAG:
                        emit_av(*pend.pop(0))
                while pend:
                    run_share()
                    emit_av(*pend.pop(0))
                normalize(t, hp, av_t)
            while done[0] < total:
                steps[done[0]]()
                done[0] += 1

        for step in qkv_round(0):
            step()
        for t in range(n_qt):
            steps = qkv_round(t + 1) if t + 1 < n_qt else []
            if t > 0:
                steps = [s for st in range(4 * (t - 1), 4 * t)
                         for s in wo_steps(st)] + steps
            attention_tile(t, steps)
        for st in range(4 * (n_qt - 1), n_st):
            for s in wo_steps(st):
                s()

    if _ENABLE_LDW_OPT:
        _fuse_ldweights(nc)
    _split_excess_waits(nc)
    salt = mybir.InstNoOp(name=f"salt_{_CFG_SALT}", ins=[], outs=[])
    salt.engine = mybir.EngineType.SP
    nc.m.functions[0].blocks[0].instructions.insert(0, salt)
    return nc


_CACHED_NC = None


def _get_nc():
    global _CACHED_NC
    if _CACHED_NC is None:
        _CACHED_NC = build_attention_nc()
    return _CACHED_NC


def _prep_core_inputs(x, mask, w_qkv_w, w_qkv_b, w_o_w, w_o_b, core):
    b = core // 4
    hg = core % 4
    heads = [hg * HPC + h for h in range(HPC)]

    xT = np.ascontiguousarray(x[b].T).astype(NP_BF16)

    def rows(sec, h):  # q=0, k=1, v=2
        base = sec * D + h * DK
        return slice(base, base + DK)

    wqk_rows = np.concatenate(
        [w_qkv_w[rows(1, h)] for h in heads] + [w_qkv_w[rows(0, h)] for h in heads],
        axis=0,
    )  # [512, 1024]
    wqk = np.ascontiguousarray(wqk_rows.T).astype(NP_BF16)

    wv_rows = np.concatenate([w_qkv_w[rows(2, h)] for h in heads], axis=0)
    wv = np.ascontiguousarray(wv_rows.T).astype(NP_BF16)

    wo = np.ascontiguousarray(
        w_o_w[:, hg * HPC * DK:(hg + 1) * HPC * DK].T
    ).astype(NP_BF16)

    bqk = np.concatenate(
        [w_qkv_b[rows(1, h)] for h in heads] + [w_qkv_b[rows(0, h)] for h in heads]
    ).astype(np.float32)[:, None]

    # Diagonal 128x128 triangle from the provided mask tensor:
    # tri[k, q] = mask[q, k] on a diagonal block (1 iff k <= q).
    m2d = np.asarray(mask[0, 0])
    tri = m2d[0:K_BLK, 0:K_BLK].T.astype(np.float32).astype(NP_BF16)

    return {
        "xT": xT, "wqk": wqk, "wv": wv, "wo": wo,
        "bqk": bqk, "tri": tri,
    }


def kernel(x, mask, w_qkv_w, w_qkv_b, w_o_w, w_o_b, _profile=False):
    x = np.asarray(x, np.float32)
    w_qkv_w = np.asarray(w_qkv_w, np.float32)
    w_qkv_b = np.asarray(w_qkv_b, np.float32)
    w_o_w = np.asarray(w_o_w, np.float32)
    w_o_b = np.asarray(w_o_b, np.float32)

    nc = _get_nc()
    in_maps = [
        _prep_core_inputs(x, mask, w_qkv_w, w_qkv_b, w_o_w, w_o_b, core=c)
        for c in range(N_CORES)
    ]
    res = run_bass_kernel_spmd(
        nc, in_maps, core_ids=list(range(N_CORES)), trace=_profile
    )
    y = np.zeros((B, S, D), np.float32)
    for c in range(N_CORES):
        y[c // 4] += np.asarray(res.results[c]["y"], np.float32)
    # v-bias flows through w_o as a constant row: y += w_o @ b_v + b_o.
    y += (w_o_w @ w_qkv_b[2 * D:3 * D] + w_o_b)[None, None, :]
    if _profile:
        return y, res
    return y
